# revision 6
# baseline (speedup 1.0000x reference)
"""Trainium2 Bass kernel for nn_LIFNode_25202868093202.

Computes:
  1. Multi-step LIF neuron scan over T=16 steps: spikes [T, N, D]
  2. Lorentz exponential map: z_out [N, D]

Sharding: N (8192) split across 8 NeuronCores, 1024 rows each; the LIF
recurrence is sequential in T only, so this is pure data parallelism.

Per-core layout: the 1024-row shard is packed 8 rows/partition as a
[128, 4096] tile.  LIF tracks w = 2*v so each step needs only two DVE
scalar_tensor_tensor ops:
    h2 = 0.5*w + x        (h2 == 2*h;  spike iff h2 >= 2)
    w  = h2 * (h2 < 2)
The spike is emitted on the (otherwise idle) ACT engine as
sign(0.5*h2 - 1) in int8, which the host maps exactly via (sraw >= 0)
(sign==0 only when h2 == 2.0 exactly, which is a spike).
"""

import os
import sys

import numpy as np

for _p in ("/opt/trn_rl_repo", os.path.expanduser("~/.axon_site/_ro/trn_rl_repo")):
    if os.path.isdir(_p) and _p not in sys.path:
        sys.path.append(_p)

import concourse.bass as bass
import concourse.bacc as bacc
import concourse.tile as tile
from concourse import mybir
from concourse.bass_utils import run_bass_kernel_spmd

T, N, D = 16, 8192, 512
NCORES = 8
NL = N // NCORES          # 1024 rows per core
P = 128                   # SBUF partitions
A = NL // P               # 8 rows packed per partition in the LIF tile
FD = A * D                # 4096 free-dim elements
G = NL // P               # 8 expmap row-groups per core

F32 = mybir.dt.float32
I8 = mybir.dt.int8
AF = mybir.ActivationFunctionType
OP = mybir.AluOpType


def _build() -> bass.Bass:
    nc = bacc.Bacc("TRN2", target_bir_lowering=False)
    x = nc.declare_dram_parameter("x", [T, P, FD], F32, isOutput=False)
    v = nc.declare_dram_parameter("v", [NL, D], F32, isOutput=False)
    z = nc.declare_dram_parameter("z", [NL, D], F32, isOutput=False)
    s = nc.declare_dram_parameter("s", [T, P, FD], I8, isOutput=True)
    zo = nc.declare_dram_parameter("zo", [NL, D], F32, isOutput=True)

    with tile.TileContext(nc) as tc:
        with (
            tc.tile_pool(name="pp", bufs=1) as pp,
            tc.tile_pool(name="xp", bufs=3) as xp,
            tc.tile_pool(name="hp", bufs=2) as hp,
            tc.tile_pool(name="sp", bufs=3) as sp,
            tc.tile_pool(name="ep", bufs=2) as ep,
        ):
            # ---------------- LIF scan over T ----------------
            bm1 = pp.tile([P, 1], F32, name="bm1")
            nc.gpsimd.memset(bm1[:], -1.0)
            w = pp.tile([P, FD], F32, name="w")
            for t in range(T):
                xt = xp.tile([P, FD], F32, name="xt", tag="xt")
                nc.sync.dma_start(out=xt[:], in_=x[t])
                if t == 0:
                    h = xt          # w(=2v) starts at 0, so h2 = x
                else:
                    h = hp.tile([P, FD], F32, name="h", tag="h")
                    nc.vector.scalar_tensor_tensor(
                        out=h[:], in0=w[:], scalar=0.5, in1=xt[:],
                        op0=OP.mult, op1=OP.add)
                st = sp.tile([P, FD], I8, name="st", tag="st")
                nc.scalar.activation(st[:], h[:], AF.Sign, bias=bm1[:], scale=0.5)
                nc.sync.dma_start(out=s[t], in_=st[:])
                if t < T - 1:
                    nc.vector.scalar_tensor_tensor(
                        out=w[:], in0=h[:], scalar=2.0, in1=h[:],
                        op0=OP.is_lt, op1=OP.mult)

            # ---------------- Lorentz expmap ----------------
            # inner = sum(v^2) - 2*v0^2 ; nrm = sqrt(max(inner, eps))
            # z_out = cosh(nrm)*z + (sinh(nrm)/nrm)*v
            acc = pp.tile([P, G], F32, name="acc")
            v0s = pp.tile([P, G], F32, name="v0s")
            vts = []
            for g in range(G):
                vt = ep.tile([P, D], F32, name=f"vt{g}", tag=f"vt{g}", bufs=1)
                nc.sync.dma_start(out=vt[:], in_=v[g * P:(g + 1) * P, :])
                vts.append(vt)
                sq = ep.tile([P, D], F32, name="sq", tag="sq")
                nc.scalar.activation(sq[:], vt[:], AF.Square,
                                     accum_out=acc[:, g:g + 1])
                nc.scalar.copy(v0s[:, g:g + 1], vt[:, 0:1])

            v0sq = pp.tile([P, G], F32, name="v0sq")
            nc.scalar.activation(v0sq[:], v0s[:], AF.Square)
            inner = pp.tile([P, G], F32, name="inner")
            nc.vector.scalar_tensor_tensor(
                out=inner[:], in0=v0sq[:], scalar=-2.0, in1=acc[:],
                op0=OP.mult, op1=OP.add)
            innc = pp.tile([P, G], F32, name="innc")
            nc.vector.tensor_scalar_max(innc[:], inner[:], 1e-6)
            nrm = pp.tile([P, G], F32, name="nrm")
            nc.scalar.activation(nrm[:], innc[:], AF.Sqrt)
            en = pp.tile([P, G], F32, name="en")
            nc.scalar.activation(en[:], nrm[:], AF.Exp)
            enm = pp.tile([P, G], F32, name="enm")
            nc.scalar.activation(enm[:], nrm[:], AF.Exp, scale=-1.0)
            c2 = pp.tile([P, G], F32, name="c2")
            nc.vector.tensor_add(c2[:], en[:], enm[:])
            cosh = pp.tile([P, G], F32, name="cosh")
            nc.vector.tensor_scalar_mul(cosh[:], c2[:], 0.5)
            s2 = pp.tile([P, G], F32, name="s2")
            nc.vector.tensor_sub(s2[:], en[:], enm[:])
            rnrm = pp.tile([P, G], F32, name="rnrm")
            nc.vector.reciprocal(rnrm[:], nrm[:])
            sor = pp.tile([P, G], F32, name="sor")
            nc.vector.scalar_tensor_tensor(
                out=sor[:], in0=s2[:], scalar=0.5, in1=rnrm[:],
                op0=OP.mult, op1=OP.mult)

            for g in range(G):
                zt = ep.tile([P, D], F32, name="zt", tag="zt")
                nc.sync.dma_start(out=zt[:], in_=z[g * P:(g + 1) * P, :])
                tmp = ep.tile([P, D], F32, name="tmp", tag="tmp")
                nc.scalar.activation(tmp[:], vts[g][:], AF.Copy,
                                     scale=sor[:, g:g + 1])
                zot = ep.tile([P, D], F32, name="zot", tag="zot")
                nc.vector.scalar_tensor_tensor(
                    out=zot[:], in0=zt[:], scalar=cosh[:, g:g + 1], in1=tmp[:],
                    op0=OP.mult, op1=OP.add)
                nc.sync.dma_start(out=zo[g * P:(g + 1) * P, :], in_=zot[:])

    nc.compile()
    return nc


_CACHE: dict = {}


def _program() -> bass.Bass:
    if "nc" not in _CACHE:
        _CACHE["nc"] = _build()
    return _CACHE["nc"]


def _make_in_maps(x_seq, v_seq, z_seq):
    in_maps = []
    for k in range(NCORES):
        sl = slice(k * NL, (k + 1) * NL)
        in_maps.append({
            "x": np.ascontiguousarray(x_seq[:, sl, :]).reshape(T, P, FD),
            "v": np.ascontiguousarray(v_seq[sl]),
            "z": np.ascontiguousarray(z_seq[sl]),
        })
    return in_maps


def _assemble(results):
    spikes = np.empty((T, N, D), np.float32)
    zout = np.empty((N, D), np.float32)
    for k, r in enumerate(results):
        sl = slice(k * NL, (k + 1) * NL)
        # sign output: -1 below threshold, 0 exactly at threshold (spike),
        # +1 above threshold (spike)  ->  spike = (sraw >= 0)
        spikes[:, sl, :] = (r["s"].reshape(T, NL, D) >= 0).astype(np.float32)
        zout[sl] = r["zo"]
    return spikes, zout


def kernel(x_seq: np.ndarray, v_seq: np.ndarray, z_seq: np.ndarray):
    x_seq = np.asarray(x_seq, np.float32)
    v_seq = np.asarray(v_seq, np.float32)
    z_seq = np.asarray(z_seq, np.float32)
    in_maps = _make_in_maps(x_seq, v_seq, z_seq)
    res = run_bass_kernel_spmd(_program(), in_maps, list(range(NCORES)))
    return _assemble(res.results)


# revision 7
# speedup vs baseline: 6.1906x; 6.1906x over previous
"""Trainium2 Bass kernel for nn_LIFNode_25202868093202.

Computes:
  1. Multi-step LIF neuron scan over T=16 steps: spikes [T, N, D]
  2. Lorentz exponential map: z_out [N, D]

Sharding: N (8192) split across 8 NeuronCores, 1024 rows each; the LIF
recurrence is sequential in T only, so this is pure data parallelism.

Per-core layout: the 1024-row shard is packed 8 rows/partition as a
[128, 4096] tile.  LIF tracks w = 2*v so each step needs only two DVE
scalar_tensor_tensor ops:
    h2 = 0.5*w + x        (h2 == 2*h;  spike iff h2 >= 2)
    w  = h2 * (h2 < 2)
The spike is emitted on the (otherwise idle) ACT engine as
sign(0.5*h2 - 1) in int8, which the host maps exactly via (sraw >= 0)
(sign==0 only when h2 == 2.0 exactly, which is a spike).
"""

import os
import sys

import numpy as np

for _p in ("/opt/trn_rl_repo", os.path.expanduser("~/.axon_site/_ro/trn_rl_repo")):
    if os.path.isdir(_p) and _p not in sys.path:
        sys.path.append(_p)

import concourse.bass as bass
import concourse.bacc as bacc
import concourse.tile as tile
from concourse import mybir
from concourse.bass_utils import run_bass_kernel_spmd

T, N, D = 16, 8192, 512
NCORES = 8
NL = N // NCORES          # 1024 rows per core
P = 128                   # SBUF partitions
A = NL // P               # 8 rows packed per partition in the LIF tile
FD = A * D                # 4096 free-dim elements
G = NL // P               # 8 expmap row-groups per core

F32 = mybir.dt.float32
I8 = mybir.dt.int8
AF = mybir.ActivationFunctionType
OP = mybir.AluOpType


def _build(repeat: int = 1) -> bass.Bass:
    nc = bacc.Bacc("TRN2", target_bir_lowering=False)
    x = nc.declare_dram_parameter("x", [T, P, FD], F32, isOutput=False)
    v = nc.declare_dram_parameter("v", [NL, D], F32, isOutput=False)
    z = nc.declare_dram_parameter("z", [NL, D], F32, isOutput=False)
    s = nc.declare_dram_parameter("s", [T, P, FD], I8, isOutput=True)
    zo = nc.declare_dram_parameter("zo", [NL, D], F32, isOutput=True)

    with tile.TileContext(nc) as tc:
        with (
            tc.tile_pool(name="pp", bufs=1) as pp,
            tc.tile_pool(name="xp", bufs=3) as xp,
            tc.tile_pool(name="hp", bufs=2) as hp,
            tc.tile_pool(name="sp", bufs=3) as sp,
            tc.tile_pool(name="ep", bufs=2) as ep,
        ):
          for _rep in range(repeat):
            # ---------------- LIF scan over T ----------------
            bm1 = pp.tile([P, 1], F32, name="bm1")
            nc.gpsimd.memset(bm1[:], -1.0)
            w = pp.tile([P, FD], F32, name="w")
            for t in range(T):
                xt = xp.tile([P, FD], F32, name="xt", tag="xt")
                nc.sync.dma_start(out=xt[:], in_=x[t])
                if t == 0:
                    h = xt          # w(=2v) starts at 0, so h2 = x
                else:
                    h = hp.tile([P, FD], F32, name="h", tag="h")
                    nc.vector.scalar_tensor_tensor(
                        out=h[:], in0=w[:], scalar=0.5, in1=xt[:],
                        op0=OP.mult, op1=OP.add)
                st = sp.tile([P, FD], I8, name="st", tag="st")
                nc.scalar.activation(st[:], h[:], AF.Sign, bias=bm1[:], scale=0.5)
                nc.sync.dma_start(out=s[t], in_=st[:])
                if t < T - 1:
                    nc.vector.scalar_tensor_tensor(
                        out=w[:], in0=h[:], scalar=2.0, in1=h[:],
                        op0=OP.is_lt, op1=OP.mult)

            # ---------------- Lorentz expmap ----------------
            # inner = sum(v^2) - 2*v0^2 ; nrm = sqrt(max(inner, eps))
            # z_out = cosh(nrm)*z + (sinh(nrm)/nrm)*v
            acc = pp.tile([P, G], F32, name="acc")
            v0s = pp.tile([P, G], F32, name="v0s")
            vts = []
            for g in range(G):
                vt = ep.tile([P, D], F32, name=f"vt{g}", tag=f"vt{g}", bufs=1)
                nc.sync.dma_start(out=vt[:], in_=v[g * P:(g + 1) * P, :])
                vts.append(vt)
                sq = ep.tile([P, D], F32, name="sq", tag="sq")
                nc.scalar.activation(sq[:], vt[:], AF.Square,
                                     accum_out=acc[:, g:g + 1])
                nc.scalar.copy(v0s[:, g:g + 1], vt[:, 0:1])

            v0sq = pp.tile([P, G], F32, name="v0sq")
            nc.scalar.activation(v0sq[:], v0s[:], AF.Square)
            inner = pp.tile([P, G], F32, name="inner")
            nc.vector.scalar_tensor_tensor(
                out=inner[:], in0=v0sq[:], scalar=-2.0, in1=acc[:],
                op0=OP.mult, op1=OP.add)
            innc = pp.tile([P, G], F32, name="innc")
            nc.vector.tensor_scalar_max(innc[:], inner[:], 1e-6)
            nrm = pp.tile([P, G], F32, name="nrm")
            nc.scalar.activation(nrm[:], innc[:], AF.Sqrt)
            en = pp.tile([P, G], F32, name="en")
            nc.scalar.activation(en[:], nrm[:], AF.Exp)
            enm = pp.tile([P, G], F32, name="enm")
            nc.scalar.activation(enm[:], nrm[:], AF.Exp, scale=-1.0)
            c2 = pp.tile([P, G], F32, name="c2")
            nc.vector.tensor_add(c2[:], en[:], enm[:])
            cosh = pp.tile([P, G], F32, name="cosh")
            nc.vector.tensor_scalar_mul(cosh[:], c2[:], 0.5)
            s2 = pp.tile([P, G], F32, name="s2")
            nc.vector.tensor_sub(s2[:], en[:], enm[:])
            rnrm = pp.tile([P, G], F32, name="rnrm")
            nc.vector.reciprocal(rnrm[:], nrm[:])
            sor = pp.tile([P, G], F32, name="sor")
            nc.vector.scalar_tensor_tensor(
                out=sor[:], in0=s2[:], scalar=0.5, in1=rnrm[:],
                op0=OP.mult, op1=OP.mult)

            for g in range(G):
                zt = ep.tile([P, D], F32, name="zt", tag="zt")
                nc.sync.dma_start(out=zt[:], in_=z[g * P:(g + 1) * P, :])
                tmp = ep.tile([P, D], F32, name="tmp", tag="tmp")
                nc.scalar.activation(tmp[:], vts[g][:], AF.Copy,
                                     scale=sor[:, g:g + 1])
                zot = ep.tile([P, D], F32, name="zot", tag="zot")
                nc.vector.scalar_tensor_tensor(
                    out=zot[:], in0=zt[:], scalar=cosh[:, g:g + 1], in1=tmp[:],
                    op0=OP.mult, op1=OP.add)
                nc.sync.dma_start(out=zo[g * P:(g + 1) * P, :], in_=zot[:])

    nc.compile()
    return nc


_CACHE: dict = {}


def _program() -> bass.Bass:
    if "nc" not in _CACHE:
        _CACHE["nc"] = _build()
    return _CACHE["nc"]


def _make_in_maps(x_seq, v_seq, z_seq):
    in_maps = []
    for k in range(NCORES):
        sl = slice(k * NL, (k + 1) * NL)
        in_maps.append({
            "x": np.ascontiguousarray(x_seq[:, sl, :]).reshape(T, P, FD),
            "v": np.ascontiguousarray(v_seq[sl]),
            "z": np.ascontiguousarray(z_seq[sl]),
        })
    return in_maps


def _assemble(results):
    spikes = np.empty((T, N, D), np.float32)
    zout = np.empty((N, D), np.float32)
    for k, r in enumerate(results):
        sl = slice(k * NL, (k + 1) * NL)
        # sign output: -1 below threshold, 0 exactly at threshold (spike),
        # +1 above threshold (spike)  ->  spike = (sraw >= 0)
        spikes[:, sl, :] = (r["s"].reshape(T, NL, D) >= 0).astype(np.float32)
        zout[sl] = r["zo"]
    return spikes, zout


def kernel(x_seq: np.ndarray, v_seq: np.ndarray, z_seq: np.ndarray):
    x_seq = np.asarray(x_seq, np.float32)
    v_seq = np.asarray(v_seq, np.float32)
    z_seq = np.asarray(z_seq, np.float32)
    in_maps = _make_in_maps(x_seq, v_seq, z_seq)
    res = run_bass_kernel_spmd(_program(), in_maps, list(range(NCORES)))
    return _assemble(res.results)
